# revision 19
# baseline (speedup 1.0000x reference)
"""Multi-head attention (B=4, T=2048, D=1024, H=16) on 8 trn2 NeuronCores.

Sharding: core c = 2*b + g handles batch b (4) x head-group g (2, 8 heads each).
Data-parallel over batch, tensor-parallel over heads in the QKV/output
projections; the output-projection partial sums of the two head-groups of each
batch are reduced on the host (cheap 33MB fp32 add) instead of an on-device
all-reduce.

Per-core kernel (all matmuls in float32r -- full fp32 storage, ~1e-4 matmul
rounding, full PE rate for moving dim >= 256), software-pipelined over 512-token
slices t so the PE-heavy projections of slice t+1 overlap the ACT-heavy
attention of slice t (causality means attention for queries in slice t only
needs K/V of slices <= t):
  for t: QT_t/KT_t = Wq/k @ xT_t (+bias), V_t = x_t @ Wv^T packed as [V|1|0|V]
         attention in the S^T layout: ST[k,q] blocks -> exp (ACT, scale=1/8)
         -> mask-block handling at 128x128 granularity derived from the actual
         mask -> AV with lhsT=[V|ones] giving the softmax denominators free ->
         normalize via DVE reciprocal + gpsimd partition_broadcast
         YT_t = Wo_g @ heads_t (partial; host adds the 2 group partials + bo).
"""
import hashlib
import ml_dtypes
import numpy as np

B, T, D, H = 4, 2048, 1024, 16
DK = D // H          # 64
P = 128
KTD = D // P         # 8 k-tiles over dmodel
NPAIR = 4            # head pairs per group
NQT = 4              # q tiles of 512
NKB = T // P         # 16 key blocks
NSB = T // P         # 16 q subblocks of 128

_CACHE = {}


def _mask_meta(mask):
    """Classify 128x128 blocks of mask[q, k]: per (qt, kb) the computed q-span
    and the zero/mixed subblocks inside it. Returns meta + packed transposed
    mixed blocks [128, nmix*128] float32."""
    m = np.asarray(mask).reshape(T, T)
    state = np.zeros((NSB, NKB), dtype=np.int8)  # rows: q subblock, cols: k block
    for qs in range(NSB):
        for kb in range(NKB):
            blk = m[qs * P:(qs + 1) * P, kb * P:(kb + 1) * P]
            if not blk.any():
                state[qs, kb] = 0
            elif blk.all():
                state[qs, kb] = 1
            else:
                state[qs, kb] = 2
    mixed_blocks = []
    meta = []  # per qt: list of (kb, qs0, zero_list, mixed_list)
    for qt in range(NQT):
        row = []
        for kb in range(NKB):
            sub = state[4 * qt:4 * qt + 4, kb]
            if not (sub != 0).any():
                continue
            qs0 = int(np.argmax(sub != 0))
            zeros = [int(q) for q in range(qs0, 4) if sub[q] == 0]
            mixed = []
            for q in range(qs0, 4):
                if sub[q] == 2:
                    qs = 4 * qt + q
                    blkT = m[qs * P:(qs + 1) * P, kb * P:(kb + 1) * P].T
                    mixed.append((q, len(mixed_blocks)))
                    mixed_blocks.append(blkT.astype(np.float32))
            row.append((kb, qs0, zeros, mixed))
        assert row and row[0][1] == 0, "first computed block must span full q tile"
        assert max(kb for kb, *_ in row) < 4 * (qt + 1), \
            "mask lets queries attend to later slices; pipelined schedule invalid"
        meta.append(row)
    nmix = len(mixed_blocks)
    if nmix:
        mm = np.concatenate(mixed_blocks, axis=1)  # [128, nmix*128]
    else:
        mm = np.zeros((P, P), np.float32)
    # the device-side mask tile is bf16; values are exactly 0/1
    return meta, np.ascontiguousarray(mm.astype(ml_dtypes.bfloat16)), nmix


def _build(meta, nmix, W=1, cfg=None):
    import contextlib
    import concourse.bacc as bacc
    import concourse.mybir as mybir
    import concourse.tile as tile
    from concourse.dt import dt

    cfg = dict(qt=2, ot=2, ata=3, sAB=1, pA=2, oab=2, yev=1, wod=3) | (cfg or {})
    f32, f32r, bf16 = dt.float32, dt.float32r, dt.bfloat16
    AF = mybir.ActivationFunctionType
    ALU = mybir.AluOpType

    nc = bacc.Bacc("TRN2", target_bir_lowering=False)
    xT = nc.dram_tensor("xT", [D, T], mybir.dt.float32r, kind="ExternalInput")
    wq = nc.dram_tensor("wq", [D, 4 * P], mybir.dt.float32r, kind="ExternalInput")
    wk = nc.dram_tensor("wk", [D, 4 * P], mybir.dt.float32r, kind="ExternalInput")
    wv = nc.dram_tensor("wv", [D, 4 * P], mybir.dt.float32r, kind="ExternalInput")
    wo = nc.dram_tensor("wo", [4 * P, D], mybir.dt.float32r, kind="ExternalInput")
    bq_d = nc.dram_tensor("bq", [P, NPAIR], mybir.dt.float32, kind="ExternalInput")
    bk_d = nc.dram_tensor("bk", [P, NPAIR], mybir.dt.float32, kind="ExternalInput")
    bv_d = nc.dram_tensor("bv", [P, NPAIR], mybir.dt.float32, kind="ExternalInput")
    mm_d = nc.dram_tensor("mm", [P, max(1, nmix) * P], mybir.dt.bfloat16,
                          kind="ExternalInput")
    vp_d = nc.dram_tensor("vp", [P, 64], mybir.dt.float32r, kind="ExternalInput")
    yt = nc.dram_tensor("yt", [D, T], mybir.dt.float32, kind="ExternalOutput")

    with tile.TileContext(nc) as tc:
        with tc.tile_pool(name="sb", bufs=1) as sb, \
             tc.tile_pool(name="ps", bufs=1, space="PSUM") as ps:
            kt_sb = sb.tile([P, NPAIR, T], f32r, tag="kt")
            v4 = sb.tile([P, NKB, NPAIR, 130], f32r, tag="v4")
            wq_sb = sb.tile([P, KTD, 4 * P], f32r, tag="wq")
            wk_sb = sb.tile([P, KTD, 4 * P], f32r, tag="wk")
            wv_sb = sb.tile([P, KTD, 4 * P], f32r, tag="wv")
            bq_t = sb.tile([P, NPAIR], f32, tag="bq")
            bk_t = sb.tile([P, NPAIR], f32, tag="bk")
            bv_t = sb.tile([P, NPAIR], f32, tag="bv")
            mm_sb = sb.tile([P, max(1, nmix), P], bf16, tag="mm")
            zero_t = sb.tile([P, P], f32, tag="zero")

            loop_ctx = tc.For_i(0, W, 1) if W > 1 else contextlib.nullcontext()
            with loop_ctx:
                # compute-critical loads first: wv halves + wq/wk, then the
                # small constants (the first V matmuls only need wv + xt(0))
                wvr = wv[:].rearrange("(kt p) n -> p kt n", p=P)
                nc.sync.dma_start(wv_sb[:, 0:KTD // 2], wvr[:, 0:KTD // 2])
                nc.sync.dma_start(wv_sb[:, KTD // 2:], wvr[:, KTD // 2:])
                nc.sync.dma_start(wq_sb[:], wq[:].rearrange("(kt p) n -> p kt n", p=P))
                nc.sync.dma_start(wk_sb[:], wk[:].rearrange("(kt p) n -> p kt n", p=P))
                nc.sync.dma_start(bq_t[:], bq_d[:])
                nc.sync.dma_start(bk_t[:], bk_d[:])
                nc.sync.dma_start(bv_t[:], bv_d[:])
                if nmix:
                    nc.sync.dma_start(mm_sb[:], mm_d[:].rearrange(
                        "p (n q) -> p n q", q=P))
                nc.vector.memset(zero_t[:], 0.0)
                # ones lanes of the packed V tile ([V_a |1| V_b |1]), via a
                # step-0 broadcast DMA over (key block, pair)
                v4f = v4[:].rearrange("p kb pr c -> p (kb pr) c")
                for c in (64, 129):
                    nc.sync.dma_start(
                        v4f[:, :, c:c + 1],
                        vp_d[:, 0:1][:, None, :].to_broadcast([P, NKB * NPAIR, 1]))

                xTr = xT[:].rearrange("(kt p) q -> p kt q", p=P)
                ytr = yt[:].rearrange("(dm p) q -> p dm q", p=P)
                KH = KTD // 2
                for t in range(4):
                    xtl = sb.tile([P, KH, 512], f32r, tag="xtl", bufs=1)
                    xth = sb.tile([P, KH, 512], f32r, tag="xth", bufs=1)
                    tsl = slice(t * 512, (t + 1) * 512)
                    nc.sync.dma_start(xtl[:], xTr[:, 0:KH, tsl])
                    nc.sync.dma_start(xth[:], xTr[:, KH:, tsl])
                    xth_of = lambda kt: xtl[:, kt] if kt < KH else xth[:, kt - KH]
                    qt_t = sb.tile([P, NPAIR, 512], f32r, tag="qt", bufs=cfg["qt"])

                    # V for token tiles of this slice (packed [V_a |1|0| V_b])
                    for tt in range(4):
                        psv = ps.tile([P, 512], f32, tag="pA", bufs=cfg["pA"])
                        for kt in range(KTD):
                            nc.tensor.matmul(
                                psv[:], xth_of(kt)[:, tt * P:(tt + 1) * P],
                                wv_sb[:, kt, :],
                                start=(kt == 0), stop=(kt == KTD - 1))
                        gt = t * 4 + tt
                        psr = psv[:].rearrange("p (h2 d2) -> p h2 d2", d2=P)
                        nc.vector.tensor_copy(v4[:, gt, :, 0:64], psr[:, :, 0:64])
                        nc.vector.tensor_copy(v4[:, gt, :, 65:129], psr[:, :, 64:128])
                    # Q/K projections for this slice
                    for p4 in range(NPAIR):
                        for w_sb, b_t, o_sb, osl in (
                                (wq_sb, bq_t, qt_t, slice(0, 512)),
                                (wk_sb, bk_t, kt_sb, slice(t * 512, (t + 1) * 512))):
                            psq = ps.tile([P, 512], f32, tag="pA", bufs=cfg["pA"])
                            for kt in range(KTD):
                                nc.tensor.matmul(
                                    psq[:], w_sb[:, kt, p4 * P:(p4 + 1) * P],
                                    xth_of(kt)[:],
                                    start=(kt == 0), stop=(kt == KTD - 1))
                            nc.vector.tensor_tensor(
                                o_sb[:, p4, osl], psq[:],
                                b_t[:, p4:p4 + 1].to_broadcast([P, 512]), ALU.add)

                    # attention for queries of this slice (qt = t)
                    ot_t = sb.tile([P, NPAIR, 512], f32r, tag="ot", bufs=cfg["ot"])
                    row = meta[t]
                    nkb = len(row)
                    for p4 in range(NPAIR):
                        oa = ps.tile([65, 512], f32, tag="oa", bufs=cfg["oab"])
                        ob = ps.tile([65, 512], f32, tag="ob", bufs=cfg["oab"])
                        for i, (kb, qs0, zeros, mixed) in enumerate(row):
                            qo = qs0 * P
                            ksl = slice(kb * P, (kb + 1) * P)
                            sA = ps.tile([P, 512], f32, tag="sA", bufs=cfg["sAB"])
                            sB = ps.tile([P, 512], f32, tag="sB", bufs=cfg["sAB"])
                            nc.tensor.matmul(
                                sA[:, qo:], kt_sb[0:64, p4, ksl],
                                qt_t[0:64, p4, qo:], start=True, stop=True)
                            nc.tensor.matmul(
                                sB[:, qo:], kt_sb[64:128, p4, ksl],
                                qt_t[64:128, p4, qo:], start=True, stop=True)
                            ata = sb.tile([P, 512], f32r, tag="ata", bufs=cfg["ata"])
                            atb = sb.tile([P, 512], f32r, tag="atb", bufs=cfg["ata"])
                            nc.scalar.activation(ata[:, qo:], sA[:, qo:],
                                                 AF.Exp, scale=0.125)
                            nc.scalar.activation(atb[:, qo:], sB[:, qo:],
                                                 AF.Exp, scale=0.125)
                            for qz in zeros:
                                zsl = slice(qz * P, (qz + 1) * P)
                                nc.vector.tensor_copy(ata[:, zsl], zero_t[:])
                                nc.vector.tensor_copy(atb[:, zsl], zero_t[:])
                            for (qm, mi) in mixed:
                                msl = slice(qm * P, (qm + 1) * P)
                                nc.vector.tensor_tensor(
                                    ata[:, msl], ata[:, msl], mm_sb[:, mi, :], ALU.mult)
                                nc.vector.tensor_tensor(
                                    atb[:, msl], atb[:, msl], mm_sb[:, mi, :], ALU.mult)
                            nc.tensor.matmul(oa[:, qo:], v4[:, kb, p4, 0:65],
                                             ata[:, qo:],
                                             start=(i == 0), stop=(i == nkb - 1))
                            nc.tensor.matmul(ob[:, qo:], v4[:, kb, p4, 65:130],
                                             atb[:, qo:],
                                             start=(i == 0), stop=(i == nkb - 1))
                        # normalize + bv -> ot_t. Both heads' sums sit at psum
                        # row 64. partition_broadcast is only used in its
                        # HW-verified form: [1,N] at partition 0 -> full
                        # [128,N] at base 0. All DVE compute ops keep in/out
                        # operands on identical partition ranges. Head b's
                        # normalized tile is DMA-shifted to partitions 64..128
                        # for the Y contraction layout.
                        ra64 = sb.tile([65, 512], f32, tag="ra64", bufs=1)
                        nc.vector.reciprocal(ra64[64:65], oa[64:65, :])
                        rb64 = sb.tile([65, 512], f32, tag="rb64", bufs=1)
                        nc.vector.reciprocal(rb64[64:65], ob[64:65, :])
                        ra = sb.tile([1, 512], f32, tag="ra", bufs=2)
                        rb = sb.tile([1, 512], f32, tag="rb", bufs=2)
                        nc.sync.dma_start(ra[:], ra64[64:65, :])
                        nc.sync.dma_start(rb[:], rb64[64:65, :])
                        rba = sb.tile([P, 512], f32, tag="rba", bufs=1)
                        rbb = sb.tile([P, 512], f32, tag="rbb", bufs=1)
                        nc.gpsimd.partition_broadcast(rba[:], ra[:])
                        nc.gpsimd.partition_broadcast(rbb[:], rb[:])
                        tmpa = sb.tile([64, 512], f32, tag="tmpa", bufs=1)
                        nc.vector.tensor_tensor(tmpa[:], oa[0:64, :],
                                                rba[0:64], ALU.mult)
                        nc.scalar.activation(ot_t[0:64, p4, :], tmpa[:],
                                             AF.Identity, bias=bv_t[0:64, p4:p4 + 1])
                        tmpb = sb.tile([64, 512], f32, tag="tmpb", bufs=1)
                        nc.vector.tensor_tensor(tmpb[:], ob[0:64, :],
                                                rbb[0:64], ALU.mult)
                        shb = sb.tile([P, 512], f32, tag="shb", bufs=1)
                        nc.sync.dma_start(shb[64:128], tmpb[:])
                        nc.scalar.activation(ot_t[64:128, p4, :], shb[64:128],
                                             AF.Identity, bias=bv_t[64:128, p4:p4 + 1])

                    # output projection for this slice (wo streamed per tile)
                    wor = wo[:].rearrange("(kt p) n -> p kt n", p=P)
                    for dm in range(8):
                        wod = sb.tile([P, NPAIR, P], f32r, tag="wod", bufs=cfg["wod"])
                        nc.sync.dma_start(wod[:], wor[:, :, dm * P:(dm + 1) * P])
                        psy = ps.tile([P, 512], f32, tag="pA", bufs=cfg["pA"])
                        for kt in range(NPAIR):
                            nc.tensor.matmul(
                                psy[:], wod[:, kt, :],
                                ot_t[:, kt, :],
                                start=(kt == 0), stop=(kt == NPAIR - 1))
                        yev = sb.tile([P, 512], f32, tag="yev", bufs=cfg["yev"])
                        nc.vector.tensor_copy(yev[:], psy[:])
                        nc.sync.dma_start(
                            ytr[:, dm, t * 512:(t + 1) * 512], yev[:])

    nc.compile()
    return nc


def _get_nc(mask, W=1):
    key = (hashlib.sha256(np.asarray(mask).tobytes()).hexdigest(), W)
    if key not in _CACHE:
        meta, mm, nmix = _mask_meta(mask)
        _CACHE[key] = (_build(meta, nmix, W), mm)
    return _CACHE[key]


def prepare_in_maps(x, mask, Wq, Wk, Wv, bq, bk, Wo, bv, mm):
    """Per-core input dicts. core c = 2*b + g."""
    in_maps = []
    WqT = np.ascontiguousarray(Wq.T.astype(np.float32))
    WkT = np.ascontiguousarray(Wk.T.astype(np.float32))
    WvT = np.ascontiguousarray(Wv.T.astype(np.float32))
    WoT = np.ascontiguousarray(Wo.T.astype(np.float32))
    vpad = np.zeros((P, 64), np.float32)
    vpad[:, 0] = 1.0
    for b in range(B):
        xTb = np.ascontiguousarray(x[b].T.astype(np.float32))
        for g in range(2):
            sl = slice(g * 512, (g + 1) * 512)
            in_maps.append({
                "xT": xTb,
                "wq": np.ascontiguousarray(WqT[:, sl]),
                "wk": np.ascontiguousarray(WkT[:, sl]),
                "wv": np.ascontiguousarray(WvT[:, sl]),
                "wo": np.ascontiguousarray(WoT[sl, :]),
                "bq": np.ascontiguousarray(bq[sl].reshape(4, P).T.astype(np.float32)),
                "bk": np.ascontiguousarray(bk[sl].reshape(4, P).T.astype(np.float32)),
                "bv": np.ascontiguousarray(bv[sl].reshape(4, P).T.astype(np.float32)),
                "mm": mm,
                "vp": vpad,
            })
    return in_maps


def kernel(x, mask, Wq, Wk, Wv, Wo, bq, bk, bv, bo):
    from concourse.bass_utils import run_bass_kernel_spmd
    x = np.asarray(x, dtype=np.float32)
    nc, mm = _get_nc(mask)
    in_maps = prepare_in_maps(x, mask, np.asarray(Wq), np.asarray(Wk),
                              np.asarray(Wv), np.asarray(bq), np.asarray(bk),
                              np.asarray(Wo), np.asarray(bv), mm)
    res = run_bass_kernel_spmd(nc, in_maps, core_ids=list(range(8)))
    out = np.empty((B, T, D), np.float32)
    bo32 = np.asarray(bo, dtype=np.float32)
    for b in range(B):
        ytp = res.results[2 * b]["yt"] + res.results[2 * b + 1]["yt"]
        out[b] = ytp.T + bo32
    return out
